# revision 29
# baseline (speedup 1.0000x reference)
"""Self-contained Trainium2 Bass kernel for the 2-layer GAT problem
(nn_GAT_68264210202658). v2: 4 SWDGE queues, no dst-side gathers (St
transpose trick), attention vectors folded into weights on host."""
import sys, os, time
for p in ('/opt/trn_rl_repo', '/root/.axon_site/_ro/trn_rl_repo', '/root/problem'):
    if p not in sys.path and os.path.isdir(p):
        sys.path.insert(0, p)
import numpy as np
import ml_dtypes
import concourse.bass as bass
import concourse.bacc as bacc
import concourse.mybir as mybir
from concourse import tile

F32 = mybir.dt.float32
BF16 = mybir.dt.bfloat16
I16 = mybir.dt.int16
AF = mybir.ActivationFunctionType
ALU = mybir.AluOpType
NEG_SLOPE = 0.2
NQUEUES = 4


def _r128(x):
    return (int(x) + 127) // 128 * 128


def wrap16(idx):
    """dma_gather idx layout: idx i at [i%16, i//16], replicated to 128 rows."""
    idx = np.asarray(idx, np.int16)
    n = len(idx)
    assert n % 16 == 0
    blk = idx.reshape(n // 16, 16).T  # [16, n/16]
    return np.tile(blk, (8, 1))       # [128, n/16]


class Cfg:
    def __init__(self, N, NCORES=8, IN_C=128, HID=32, HEADS=8, OUT=16):
        assert N % NCORES == 0
        self.N, self.NCORES = N, NCORES
        self.IN_C, self.HID, self.HEADS, self.OUT = IN_C, HID, HEADS, OUT
        self.HC = HID * HEADS                      # 256
        self.NLOC = N // NCORES                    # 6250
        self.NLOCP = _r128(self.NLOC)              # 6272
        self.W = self.NLOCP // 128                 # 49 windows per core
        self.NTOT = self.NLOCP * NCORES            # 50176
        # src side split (512-aligned so phase-A write batches never straddle)
        self.SPLIT = (N // 2) // 512 * 512
        assert 0 < self.SPLIT <= 32767 and N - self.SPLIT <= 32767
        # l2 table split = l2idx(SPLIT); l2idx is monotonic in src
        self.L2SPLIT = (self.SPLIT // self.NLOC) * self.NLOCP + self.SPLIT % self.NLOC
        assert self.L2SPLIT <= 32768 and self.NTOT - self.L2SPLIT <= 32767
        # node table row: [h1 (HC) | a_src1 (HEADS) | pad] in bf16, 256B-mult
        self.ROW1 = _r128(self.HC + self.HEADS)    # 384 bf16 = 768B
        self.ROW1W = self.HC + self.HEADS          # written cols
        self.ROW2 = 64                             # l2 table row (f32): [h2(16)|a_src2|pad]
        self.EXT = self.HC + 2 * self.HEADS        # 272: h1|asrc|adst


def preprocess(cfg, edge_index):
    """Bucket/pad edges. Returns per-core input dict pieces + shared meta."""
    c = cfg
    src = np.asarray(edge_index[0], np.int64)
    dst = np.asarray(edge_index[1], np.int64)
    loops = np.arange(c.N, dtype=np.int64)
    src = np.concatenate([src, loops])
    dst = np.concatenate([dst, loops])

    core = dst // c.NLOC
    dloc = dst - core * c.NLOC
    win = dloc // 128

    # per (core, window, side) edge lists
    buckets = {}
    for cc in range(c.NCORES):
        m = core == cc
        s_c, dl_c, w_c = src[m], dloc[m], win[m]
        sideB = s_c >= c.SPLIT
        for w in range(c.W):
            mw = w_c == w
            for sb in (False, True):
                mm = mw & (sideB == sb)
                buckets[(cc, w, sb)] = (s_c[mm], dl_c[mm])

    NA = [
        _r128(max(len(buckets[(cc, w, False)][0]) for cc in range(c.NCORES)))
        for w in range(c.W)
    ]
    NB = [
        _r128(max(len(buckets[(cc, w, True)][0]) for cc in range(c.NCORES)))
        for w in range(c.W)
    ]
    # avoid zero-size gathers: keep >=128 so program structure is sane
    NA = [max(n, 128) for n in NA]
    NB = [max(n, 128) for n in NB]

    # batches of 2 windows; group ordering per batch: [A_w0 | A_w1 | B_w0 | B_w1]
    batches = [tuple(range(i, min(i + 2, c.W))) for i in range(0, c.W, 2)]
    meta = {"NA": NA, "NB": NB, "batches": batches}

    # global group numbering
    gof = {}   # (w, side) -> first group index
    g = 0
    for ws in batches:
        for w in ws:
            gof[(w, 0)] = g
            g += NA[w] // 128
        for w in ws:
            gof[(w, 1)] = g
            g += NB[w] // 128
    GT = g
    meta["gof"] = gof
    meta["GT"] = GT

    def l2idx(s):
        return (s // c.NLOC) * c.NLOCP + (s % c.NLOC)

    per_core = []
    for cc in range(c.NCORES):
        iA, iB, iA2, iB2 = [], [], [], []
        dl_cols = np.full((128, GT), -1.0, np.float32)
        for ws in batches:
            for side in (0, 1):
                for w in ws:
                    s_e, dl_e = buckets[(cc, w, bool(side))]
                    n = (NA if side == 0 else NB)[w]
                    npad = n - len(s_e)
                    padv = 0 if side == 0 else c.SPLIT
                    sp = np.concatenate([s_e, np.full(npad, padv, np.int64)])
                    dlp = np.concatenate(
                        [dl_e % 128, np.full(npad, -1, np.int64)]
                    ).astype(np.float32)
                    if side == 0:
                        iA.append(sp)
                        iA2.append(l2idx(sp))
                    else:
                        iB.append(sp - c.SPLIT)
                        iB2.append(l2idx(sp) - c.L2SPLIT)
                    g0 = gof[(w, side)]
                    dl_cols[:, g0 : g0 + n // 128] = dlp.reshape(n // 128, 128).T
        iA = np.concatenate(iA)
        iB = np.concatenate(iB)
        iA2 = np.concatenate(iA2)
        iB2 = np.concatenate(iB2)
        per_core.append(
            {
                "idxA": wrap16(iA),
                "idxB": wrap16(iB),
                "idxA2": wrap16(iA2),
                "idxB2": wrap16(iB2),
                "dl": dl_cols.astype(ml_dtypes.bfloat16),
            }
        )
    return per_core, meta


def make_consts(cfg, W1, att_src1, att_dst1, b1, W2, att_src2, att_dst2, b2):
    c = cfg
    bf = ml_dtypes.bfloat16
    H, HID, HC, OUT = c.HEADS, c.HID, c.HC, c.OUT
    W1 = np.asarray(W1, np.float32)
    As = np.zeros((HC, H), np.float32)
    Ad = np.zeros((HC, H), np.float32)
    a_s1 = np.asarray(att_src1, np.float32)
    a_d1 = np.asarray(att_dst1, np.float32)
    for h in range(H):
        As[h * HID : (h + 1) * HID, h] = a_s1[h]
        Ad[h * HID : (h + 1) * HID, h] = a_d1[h]
    W1e = np.concatenate([W1, W1 @ As, W1 @ Ad], axis=1)       # [128, 272]
    w2 = np.asarray(W2, np.float32)                            # [256, 16]
    W2e = np.concatenate(
        [w2,
         w2 @ np.asarray(att_src2, np.float32).reshape(OUT, 1),
         w2 @ np.asarray(att_dst2, np.float32).reshape(OUT, 1)],
        axis=1,
    )                                                          # [256, 18]
    cb = np.concatenate(
        [W1e] + [W2e[k * 128 : (k + 1) * 128, :] for k in range(HC // 128)],
        axis=1,
    ).astype(bf)                                               # [128, 272+2*18]
    rep = lambda v: np.tile(np.asarray(v, np.float32).reshape(1, -1), (128, 1))
    cf = np.concatenate([rep(b1), rep(b2)], axis=1).astype(np.float32)
    return cb, cf


def build(cfg, meta, phases='ABCDE', debug=False):
    c = cfg
    NA, NB, batches, gof = meta["NA"], meta["NB"], meta["batches"], meta["gof"]
    GT = meta["GT"]
    H, HID, HC, OUT = c.HEADS, c.HID, c.HC, c.OUT
    KF = c.HC // 128  # feature chunks for layer-2 matmul (2)
    EXT = c.EXT       # 272
    O2 = OUT + 2      # 18: h2|asrc2|adst2

    nc = bacc.Bacc(None, target_bir_lowering=False, debug=False,
                   num_swdge_queues=NQUEUES)

    xT = nc.dram_tensor("xT", [128, c.NTOT], BF16, kind="ExternalInput")
    xTL = nc.dram_tensor("xTL", [128, c.NLOCP], BF16, kind="ExternalInput")
    cbf = nc.dram_tensor("cbf", [128, EXT + KF * O2], BF16, kind="ExternalInput")
    cf32 = nc.dram_tensor("cf32", [128, HC + OUT], F32, kind="ExternalInput")
    sumA, sumB = sum(NA), sum(NB)
    idxA = nc.dram_tensor("idxA", [128, sumA // 16], I16, kind="ExternalInput")
    idxB = nc.dram_tensor("idxB", [128, sumB // 16], I16, kind="ExternalInput")
    idxA2 = nc.dram_tensor("idxA2", [128, sumA // 16], I16, kind="ExternalInput")
    idxB2 = nc.dram_tensor("idxB2", [128, sumB // 16], I16, kind="ExternalInput")
    dlin = nc.dram_tensor("dl", [128, GT], BF16, kind="ExternalInput")
    out = nc.dram_tensor("out", [c.NLOCP, OUT], F32, kind="ExternalOutput")

    htabA = nc.dram_tensor("htabA", [c.SPLIT, c.ROW1], BF16)
    htabB = nc.dram_tensor("htabB", [c.NTOT - c.SPLIT, c.ROW1], BF16)
    l2loc = nc.dram_tensor("l2loc", [c.NLOCP, c.ROW2], F32)
    l2glob = nc.dram_tensor("l2glob", [c.NTOT, c.ROW2], F32,
                            addr_space="Shared")
    l2globB = nc.dram_tensor("l2globB", [c.NTOT - c.L2SPLIT, c.ROW2], F32)
    if debug:
        ndbg = min(c.SPLIT, c.NLOCP)
        dbgH = nc.dram_tensor("dbgH", [ndbg, c.ROW1W], BF16, kind="ExternalOutput")
        dbgL = nc.dram_tensor("dbgL", [c.NLOCP, OUT + 1], F32, kind="ExternalOutput")
        dbgD2 = nc.dram_tensor("dbgD2", [c.NLOCP, 1], BF16, kind="ExternalOutput")

    ctx_lp = nc.allow_low_precision("bf16 tables are intentional")
    ctx_lp.__enter__()
    with tile.TileContext(nc) as tc:
        with tc.tile_pool(name="consts", bufs=1) as cpool:
            CB = cpool.tile([128, EXT + KF * O2], BF16)
            CF = cpool.tile([128, HC + OUT], F32)
            nc.sync.dma_start(CB[:], cbf[:])
            nc.sync.dma_start(CF[:], cf32[:])
            W1e = CB[:, 0:EXT]
            W2s = [CB[:, EXT + k * O2 : EXT + (k + 1) * O2] for k in range(KF)]
            B1 = CF[:, 0:HC]
            B2 = CF[:, HC : HC + OUT]

            iota_i = cpool.tile([128, 128], I16)
            nc.gpsimd.iota(iota_i[:], [[1, 128]], channel_multiplier=0)
            iota_b = cpool.tile([128, 128], BF16)
            nc.vector.tensor_copy(iota_b[:], iota_i[:])
            iotac_i = cpool.tile([128, 1], I16)
            nc.gpsimd.iota(iotac_i[:], [[1, 1]], channel_multiplier=1)
            iotac_f = cpool.tile([128, 1], F32)
            nc.vector.tensor_copy(iotac_f[:], iotac_i[:])
            iota_f = cpool.tile([128, 128], F32)
            nc.vector.tensor_copy(iota_f[:], iota_i[:])
            ident = cpool.tile([128, 128], BF16)
            nc.vector.tensor_scalar(ident[:], iota_f[:], iotac_f[:], None, ALU.is_equal)

            AD1sb = cpool.tile([128, c.W, H], BF16)   # a_dst1 per (lane, win)
            dst2_acc = cpool.tile([128, c.W], BF16)   # a_dst2 per (lane, win)

            def gather_chunked(out_tile, g_off, table, idx_tile, i_off, n, elem):
                """dma_gather in <=1024-idx chunks. queue_num is reassigned
                post-compile to match the scheduler's DMASW lane striping."""
                done = 0
                while done < n:
                    k = min(1024, n - done)
                    nc.gpsimd.dma_gather(
                        out_tile[:, (g_off + done) // 128 : (g_off + done + k) // 128, :],
                        table[:, :], idx_tile[:, (i_off + done) // 16 : (i_off + done + k) // 16],
                        k, k, elem, queue_num=0)
                    done += k

            # ---------------- Phase A: global h1|asrc table ----------------
            TB = 4  # node tiles per batch
            nbat = c.NTOT // (128 * TB) if 'A' in phases else 0
            assert c.NTOT % (128 * TB) == 0
            with (
                tc.tile_pool(name="pa", bufs=3) as pa,
                tc.tile_pool(name="ppa", bufs=2, space="PSUM") as ppa,
            ):
                def node_tile_batch(xsrc, t0, nt):
                    xt_ = pa.tile([128, nt, 128], BF16, tag="xt")
                    nc.sync.dma_start(
                        xt_[:], xsrc[:, 128 * t0 : 128 * (t0 + nt)]
                        .rearrange("p (g n) -> p g n", g=nt)
                    )
                    # 512-f32 group stride: matmul outputs (272 f32 = 1088B)
                    # must not straddle 2KB PSUM bank boundaries
                    hp_ = ppa.tile([128, nt, 512], F32, tag="h1p")
                    for j in range(nt):
                        nc.tensor.matmul(
                            hp_[:, j, 0:EXT], xt_[:, j, :], W1e,
                            start=True, stop=True,
                        )
                    return hp_

                for b in range(nbat):
                    hp_ = node_tile_batch(xT, b * TB, TB)
                    row = pa.tile([128, TB, c.ROW1W], BF16, tag="row")
                    nc.scalar.copy(row[:], hp_[:, :, 0 : c.ROW1W])
                    r0 = 128 * TB * b
                    r1 = 128 * TB * (b + 1)
                    assert r1 <= c.SPLIT or r0 >= c.SPLIT, "batch straddles SPLIT"
                    tgt = (htabA[r0:r1, 0:c.ROW1W] if r1 <= c.SPLIT
                           else htabB[r0 - c.SPLIT : r1 - c.SPLIT, 0:c.ROW1W])
                    nc.sync.dma_start(
                        tgt.rearrange("(g p) c -> p g c", p=128), row[:])
                # A-local: a_dst1 for own nodes -> SBUF resident
                t = 0 if 'A' in phases else c.W
                while t < c.W:
                    nt = min(TB, c.W - t)
                    hp_ = node_tile_batch(xTL, t, nt)
                    nc.vector.tensor_copy(
                        AD1sb[:, t : t + nt, :], hp_[:, :, HC + H : HC + 2 * H])
                    t += nt

            # ---------------- Phases B + C: layer-1 windows ----------------
            bc_batches = batches if 'B' in phases else []
            oA = oB = 0
            with (
                tc.tile_pool(name="pb", bufs=2) as pb,
                tc.tile_pool(name="pbs", bufs=3) as pbs,
                tc.tile_pool(name="pst", bufs=4) as pst,
                tc.tile_pool(name="ppb", bufs=2, space="PSUM") as ppb,
                tc.tile_pool(name="pptp", bufs=1, space="PSUM") as pptp,
                tc.tile_pool(name="ppad", bufs=1, space="PSUM") as ppad,
                tc.tile_pool(name="ppc", bufs=1, space="PSUM") as ppc,
            ):
                for bi, ws in enumerate(bc_batches):
                    nA = sum(NA[w] for w in ws)
                    nB = sum(NB[w] for w in ws)
                    nE = nA + nB
                    gb = nE // 128          # groups this batch
                    g0 = gof[(ws[0], 0)]    # first global group
                    # group -> window map for this batch
                    g2w = [None] * gb
                    for w in ws:
                        for side in (0, 1):
                            n = (NA if side == 0 else NB)[w]
                            for g in range(gof[(w, side)] - g0,
                                           gof[(w, side)] - g0 + n // 128):
                                g2w[g] = w
                    # idx tiles
                    tA = pb.tile([128, nA // 16], I16, tag="tA")
                    tB = pb.tile([128, nB // 16], I16, tag="tB")
                    nc.sync.dma_start(tA[:], idxA[:, oA // 16 : (oA + nA) // 16])
                    nc.sync.dma_start(tB[:], idxB[:, oB // 16 : (oB + nB) // 16])
                    dl = pb.tile([128, gb], BF16, tag="dl")
                    nc.sync.dma_start(dl[:], dlin[:, g0 : g0 + gb])
                    # gathers (src rows only)
                    G = pb.tile([128, gb, c.ROW1], BF16, tag="G")
                    gather_chunked(G, 0, htabA, tA, 0, nA, c.ROW1)
                    gather_chunked(G, nA, htabB, tB, 0, nB, c.ROW1)
                    # one-hot S for all groups (bf16, one DVE op)
                    S_all = pb.tile([128, gb, 128], BF16, tag="S")
                    nc.vector.tensor_tensor(
                        S_all[:],
                        iota_b[:].unsqueeze(1).broadcast_to([128, gb, 128]),
                        dl[:].unsqueeze(2).broadcast_to([128, gb, 128]),
                        ALU.is_equal)
                    # per-group St = S^T; adst_e = St^T @ AD1sb[win]
                    adst_all = ppad.tile([128, gb, H], F32, tag="adst")
                    for g in range(gb):
                        tp = pptp.tile([128, 128], BF16, tag="tp")
                        nc.tensor.transpose(tp[:], S_all[:, g, :], ident[:])
                        Stg = pst.tile([128, 128], BF16, tag="St")
                        if g % 2 == 0:
                            nc.scalar.copy(Stg[:], tp[:])
                        else:
                            nc.vector.tensor_copy(Stg[:], tp[:])
                        nc.tensor.matmul(
                            adst_all[:, g, :], Stg[:], AD1sb[:, g2w[g], :],
                            start=True, stop=True)
                    # logits
                    lg = pbs.tile([128, gb, H], F32, tag="lg")
                    nc.vector.tensor_add(lg[:], G[:, :, HC : HC + H], adst_all[:])
                    nc.vector.scalar_tensor_tensor(
                        lg[:], lg[:], NEG_SLOPE, lg[:], ALU.mult, ALU.max)
                    ex = pbs.tile([128, gb, H], BF16, tag="ex")
                    nc.scalar.activation(ex[:], lg[:], AF.Exp)
                    # rhs = [msg | ex]
                    rhs = pb.tile([128, gb, HC + H], BF16, tag="rhs")
                    nc.vector.tensor_tensor(
                        rhs[:, :, 0:HC].rearrange("p g (h c) -> p g h c", h=H),
                        G[:, :, 0:HC].rearrange("p g (h c) -> p g h c", h=H),
                        ex[:].unsqueeze(3).broadcast_to([128, gb, H, HID]),
                        ALU.mult)
                    nc.scalar.copy(rhs[:, :, HC : HC + H], ex[:])
                    # per-batch reduce + normalize + elu (both windows at once)
                    nw = len(ws)
                    acc2 = ppb.tile([128, nw, 512], F32, tag="acc")
                    for wi, w in enumerate(ws):
                        glist = (
                            list(range(gof[(w, 0)] - g0, gof[(w, 0)] - g0 + NA[w] // 128))
                            + list(range(gof[(w, 1)] - g0, gof[(w, 1)] - g0 + NB[w] // 128))
                        )
                        for i, g in enumerate(glist):
                            nc.tensor.matmul(
                                acc2[:, wi, 0 : HC + H], S_all[:, g, :], rhs[:, g, :],
                                start=(i == 0), stop=(i == len(glist) - 1))
                    den = pbs.tile([128, nw, H], F32, tag="den")
                    nc.vector.tensor_scalar(
                        den[:], acc2[:, :, HC : HC + H], 1e-30, None, ALU.max)
                    rec = pbs.tile([128, nw, H], F32, tag="rec")
                    nc.vector.reciprocal(rec[:], den[:])
                    o1 = pbs.tile([128, nw, HC], F32, tag="o1")
                    nc.vector.tensor_tensor(
                        o1[:].rearrange("p w (h c) -> p w h c", h=H),
                        acc2[:, :, 0:HC].rearrange("p w (h c) -> p w h c", h=H),
                        rec[:].unsqueeze(3).broadcast_to([128, nw, H, HID]),
                        ALU.mult)
                    nc.vector.tensor_add(
                        o1[:], o1[:], B1.unsqueeze(1).broadcast_to([128, nw, HC]))
                    # elu: hp = max(o1,0) + exp(min(o1,0)) - 1
                    t1 = pbs.tile([128, nw, HC], F32, tag="t1")
                    nc.vector.tensor_scalar(t1[:], o1[:], 0.0, None, ALU.min)
                    e1 = pbs.tile([128, nw, HC], F32, tag="e1")
                    nc.scalar.activation(e1[:], t1[:], AF.Exp)
                    nc.vector.tensor_scalar(t1[:], o1[:], 0.0, None, ALU.max)
                    hp = pbs.tile([128, nw, HC], BF16, tag="hp")
                    nc.vector.scalar_tensor_tensor(
                        hp[:], e1[:], -1.0, t1[:], ALU.add, ALU.add)
                    # ---- phase C for this batch ----
                    if 'C' in phases:
                        for wi, w in enumerate(ws):
                            tp2 = ppc.tile([128, KF, 128], BF16, tag="tp2")
                            for k in range(KF):
                                nc.tensor.transpose(
                                    tp2[:, k, :],
                                    hp[:, wi, 128 * k : 128 * (k + 1)], ident[:])
                            tpb = pbs.tile([128, KF, 128], BF16, tag="tpb")
                            nc.scalar.copy(tpb[:], tp2[:])
                            h2p = ppc.tile([128, O2], F32, tag="h2p")
                            for k in range(KF):
                                nc.tensor.matmul(
                                    h2p[:], tpb[:, k, :], W2s[k],
                                    start=(k == 0), stop=(k == KF - 1))
                            row2 = pbs.tile([128, OUT + 1], F32, tag="row2")
                            nc.scalar.copy(row2[:], h2p[:, 0 : OUT + 1])
                            nc.sync.dma_start(
                                l2loc[128 * w : 128 * (w + 1), 0 : OUT + 1]
                                .rearrange("(g p) c -> p g c", p=128)[:, 0, :],
                                row2[:])
                            nc.vector.tensor_copy(
                                dst2_acc[:, w : w + 1], h2p[:, OUT + 1 : OUT + 2])
                    oA += nA
                    oB += nB

            if debug:
                nc.sync.dma_start(dbgH[:], htabA[0:ndbg, 0:c.ROW1W])
                nc.sync.dma_start(dbgL[:], l2loc[:, 0 : OUT + 1])
                nc.sync.dma_start(
                    dbgD2[:].rearrange("(w p) c -> p w c", p=128),
                    dst2_acc[:].unsqueeze(2))

            # ---------------- Phase D: allgather l2 ----------------
            if 'D' in phases:
                nc.gpsimd.collective_compute(
                    "AllGather", ALU.bypass,
                    replica_groups=[list(range(c.NCORES))],
                    ins=[l2loc[:].opt()], outs=[l2glob[:].opt()])
            elif 'E' in phases:
                # debug: bypass collective, local copy only (wrong values off-core)
                nc.sync.dma_start(l2glob[0 : c.NLOCP, :], l2loc[:])
            if 'D' in phases or 'E' in phases:
                nc.sync.dma_start(l2globB[:], l2glob[c.L2SPLIT : c.NTOT, :])

            # ---------------- Phase E: layer-2 windows ----------------
            oA = oB = 0
            e_batches = batches if 'E' in phases else []
            with (
                tc.tile_pool(name="pe", bufs=2) as pe,
                tc.tile_pool(name="pes", bufs=3) as pes,
                tc.tile_pool(name="pst2", bufs=4) as pst2,
                tc.tile_pool(name="ppe", bufs=3, space="PSUM") as ppe,
                tc.tile_pool(name="pptp2", bufs=2, space="PSUM") as pptp2,
                tc.tile_pool(name="ppad2", bufs=1, space="PSUM") as ppad2,
            ):
                for ws in e_batches:
                    nA = sum(NA[w] for w in ws)
                    nB = sum(NB[w] for w in ws)
                    nE = nA + nB
                    gb = nE // 128
                    g0 = gof[(ws[0], 0)]
                    g2w = [None] * gb
                    for w in ws:
                        for side in (0, 1):
                            n = (NA if side == 0 else NB)[w]
                            for g in range(gof[(w, side)] - g0,
                                           gof[(w, side)] - g0 + n // 128):
                                g2w[g] = w
                    tA = pe.tile([128, nA // 16], I16, tag="tA")
                    tB = pe.tile([128, nB // 16], I16, tag="tB")
                    nc.sync.dma_start(tA[:], idxA2[:, oA // 16 : (oA + nA) // 16])
                    nc.sync.dma_start(tB[:], idxB2[:, oB // 16 : (oB + nB) // 16])
                    dl = pe.tile([128, gb], BF16, tag="dl")
                    nc.sync.dma_start(dl[:], dlin[:, g0 : g0 + gb])
                    G2 = pe.tile([128, gb, c.ROW2], F32, tag="G2")
                    gather_chunked(G2, 0, l2glob[0 : c.L2SPLIT, :], tA, 0, nA, c.ROW2)
                    gather_chunked(G2, nA, l2globB, tB, 0, nB, c.ROW2)
                    S_all = pe.tile([128, gb, 128], BF16, tag="S2")
                    nc.vector.tensor_tensor(
                        S_all[:],
                        iota_b[:].unsqueeze(1).broadcast_to([128, gb, 128]),
                        dl[:].unsqueeze(2).broadcast_to([128, gb, 128]),
                        ALU.is_equal)
                    adst_all = ppad2.tile([128, gb, 1], F32, tag="adst2")
                    for g in range(gb):
                        tp = pptp2.tile([128, 128], BF16, tag="tp")
                        nc.tensor.transpose(tp[:], S_all[:, g, :], ident[:])
                        Stg = pst2.tile([128, 128], BF16, tag="St")
                        if g % 2 == 0:
                            nc.scalar.copy(Stg[:], tp[:])
                        else:
                            nc.vector.tensor_copy(Stg[:], tp[:])
                        nc.tensor.matmul(
                            adst_all[:, g, :], Stg[:],
                            dst2_acc[:, g2w[g] : g2w[g] + 1],
                            start=True, stop=True)
                    lg = pes.tile([128, gb, 1], F32, tag="lg2")
                    nc.vector.tensor_add(
                        lg[:], G2[:, :, OUT : OUT + 1], adst_all[:])
                    nc.vector.scalar_tensor_tensor(
                        lg[:], lg[:], NEG_SLOPE, lg[:], ALU.mult, ALU.max)
                    ex = pes.tile([128, gb, 1], BF16, tag="ex2")
                    nc.scalar.activation(ex[:], lg[:], AF.Exp)
                    rhs = pe.tile([128, gb, OUT + 1], BF16, tag="rhs2")
                    nc.vector.tensor_tensor(
                        rhs[:, :, 0:OUT], G2[:, :, 0:OUT],
                        ex[:].broadcast_to([128, gb, OUT]), ALU.mult)
                    nc.scalar.copy(rhs[:, :, OUT : OUT + 1], ex[:])
                    nw = len(ws)
                    acc2 = ppe.tile([128, nw, 32], F32, tag="acc2")
                    for wi, w in enumerate(ws):
                        glist = (
                            list(range(gof[(w, 0)] - g0, gof[(w, 0)] - g0 + NA[w] // 128))
                            + list(range(gof[(w, 1)] - g0, gof[(w, 1)] - g0 + NB[w] // 128))
                        )
                        for i, g in enumerate(glist):
                            nc.tensor.matmul(
                                acc2[:, wi, 0 : OUT + 1], S_all[:, g, :], rhs[:, g, :],
                                start=(i == 0), stop=(i == len(glist) - 1))
                    den = pes.tile([128, nw, 1], F32, tag="den2")
                    nc.vector.tensor_scalar(
                        den[:], acc2[:, :, OUT : OUT + 1], 1e-30, None, ALU.max)
                    rec = pes.tile([128, nw, 1], F32, tag="rec2")
                    nc.vector.reciprocal(rec[:], den[:])
                    o2 = pes.tile([128, nw, OUT], F32, tag="o2")
                    nc.vector.tensor_tensor(
                        o2[:], acc2[:, :, 0:OUT],
                        rec[:].broadcast_to([128, nw, OUT]), ALU.mult)
                    nc.vector.tensor_add(
                        o2[:], o2[:], B2.unsqueeze(1).broadcast_to([128, nw, OUT]))
                    nc.sync.dma_start(
                        out[128 * ws[0] : 128 * (ws[0] + nw), :]
                        .rearrange("(g p) c -> p g c", p=128),
                        o2[:])
                    oA += nA
                    oB += nB
    ctx_lp.__exit__(None, None, None)
    nc.compile()
    # Stripe SWDGE queues to match the tile scheduler's DMASW-lane round-robin
    # (lane = i % 8 over Pool DMA insts in scheduled order; each lane must
    # always pair with one queue, so queue = lane % NQUEUES = i % NQUEUES).
    from concourse.tile_scheduler import DMAInst
    i = 0
    for bb in nc.m.functions[0].blocks:
        for inst in bb.instructions:
            if isinstance(inst, DMAInst) and inst.engine == mybir.EngineType.Pool:
                inst.queue_num = i % NQUEUES
                i += 1
    return nc


def make_inputs(cfg, x, per_core, consts):
    c = cfg
    bf = ml_dtypes.bfloat16
    cb, cf = consts
    xTfull = np.zeros((128, c.NTOT), bf)
    xTfull[:, 0 : c.N] = np.ascontiguousarray(np.asarray(x, np.float32).T).astype(bf)
    in_maps = []
    for cc in range(c.NCORES):
        xtl = np.zeros((128, c.NLOCP), bf)
        xtl[:, 0 : c.NLOC] = xTfull[:, cc * c.NLOC : (cc + 1) * c.NLOC]
        m = per_core[cc]
        in_maps.append(
            {
                "xT": xTfull,
                "xTL": xtl,
                "cbf": cb,
                "cf32": cf,
                "idxA": m["idxA"],
                "idxB": m["idxB"],
                "idxA2": m["idxA2"],
                "idxB2": m["idxB2"],
                "dl": m["dl"],
            }
        )
    return in_maps


def postprocess(cfg, results):
    c = cfg
    outs = [results[cc]["out"][0 : c.NLOC, :] for cc in range(c.NCORES)]
    return np.concatenate(outs, axis=0)


LAST_EXEC_NS = None
N = 50000


def _make_runner(nc, n_cores):
    """Build the shard_map-jitted executable once (mirrors
    bass2jax.run_bass_via_pjrt) so repeated timed calls skip re-tracing."""
    import jax
    from jax.experimental.shard_map import shard_map
    from jax.sharding import Mesh, PartitionSpec
    from concourse import bass2jax, mybir
    from concourse.bass2jax import _bass_exec_p, partition_id_tensor, install_neuronx_cc_hook

    install_neuronx_cc_hook()
    partition_name = nc.partition_id_tensor.name if nc.partition_id_tensor else None
    in_names, out_names, out_avals, zero_outs, in_structs = [], [], [], [], []
    for alloc in nc.m.functions[0].allocations:
        if not isinstance(alloc, mybir.MemoryLocationSet):
            continue
        name = alloc.memorylocations[0].name
        if alloc.kind == "ExternalInput":
            if name != partition_name:
                in_names.append(name)
                in_structs.append(
                    (tuple(alloc.tensor_shape), mybir.dt.np(alloc.dtype)))
        elif alloc.kind == "ExternalOutput":
            out_names.append(name)
            shape = tuple(alloc.tensor_shape)
            dtype = mybir.dt.np(alloc.dtype)
            out_avals.append(jax.core.ShapedArray(shape, dtype))
            zero_outs.append(np.zeros(shape, dtype))
    n_params = len(in_names)
    n_outs = len(out_avals)
    all_in = list(in_names) + list(out_names)
    if partition_name is not None:
        all_in.append(partition_name)
    donate = tuple(range(n_params, n_params + n_outs))

    def _body(*args):
        operands = list(args)
        if partition_name is not None:
            operands.append(partition_id_tensor())
        return tuple(
            _bass_exec_p.bind(
                *operands,
                out_avals=tuple(out_avals),
                in_names=tuple(all_in),
                out_names=tuple(out_names),
                lowering_input_output_aliases=(),
                sim_require_finite=False,
                sim_require_nnan=False,
                nc=nc,
            )
        )

    devices = jax.devices()[:n_cores]
    mesh = Mesh(np.asarray(devices), ("core",))
    in_specs = (PartitionSpec("core"),) * (n_params + n_outs)
    out_specs = (PartitionSpec("core"),) * n_outs
    sharding = jax.sharding.NamedSharding(mesh, PartitionSpec("core"))

    def _compile():
        jt = jax.jit(
            shard_map(_body, mesh=mesh, in_specs=in_specs, out_specs=out_specs,
                      check_rep=False),
            donate_argnums=donate, keep_unused=True)
        structs = [
            jax.ShapeDtypeStruct((n_cores * s[0], *s[1:]), dt, sharding=sharding)
            for s, dt in in_structs
        ] + [
            jax.ShapeDtypeStruct((n_cores * z.shape[0], *z.shape[1:]), z.dtype,
                                 sharding=sharding)
            for z in zero_outs
        ]
        return jt.lower(*structs).compile()

    try:
        from concourse.bass2jax import fast_dispatch_compile
        sharded = fast_dispatch_compile(_compile)
    except Exception as e:
        print(f"[kernel] fast_dispatch_compile unavailable ({e}); plain jit")
        sharded = jax.jit(
            shard_map(_body, mesh=mesh, in_specs=in_specs, out_specs=out_specs,
                      check_rep=False),
            donate_argnums=donate, keep_unused=True)

    def time_loop(in_maps, R=30):
        """Chain R executions, feeding each call's donated outputs back in
        (device-resident) so per-iteration cost ~= device exec + dispatch.
        Warm up with the same call signature as the timed loop so jax
        retracing / first-donation setup stays out of the timed window."""
        import jax
        concat_in = [
            np.concatenate([np.asarray(in_maps[c][i_name]) for c in range(n_cores)], axis=0)
            for i_name in in_names
        ]
        dev_in = jax.device_put(concat_in, [sharding] * n_params)
        zeros = jax.device_put(
            [np.zeros((n_cores * z.shape[0], *z.shape[1:]), z.dtype)
             for z in zero_outs], [sharding] * n_outs)
        outs = sharded(*dev_in, *zeros)
        jax.block_until_ready(outs)
        # same-signature warmups (device-resident donated outs)
        for _ in range(3):
            outs = sharded(*dev_in, *outs)
            jax.block_until_ready(outs)
        t0 = time.time()
        for _ in range(R):
            outs = sharded(*dev_in, *outs)
        jax.block_until_ready(outs)
        return (time.time() - t0) / R

    def run(in_maps, n_iter=1):
        import jax
        concat_in = [
            np.concatenate([np.asarray(in_maps[c][i_name]) for c in range(n_cores)], axis=0)
            for i_name in in_names
        ]
        dev_in = jax.device_put(concat_in, [sharding] * n_params)
        times = []
        outs = None
        for _ in range(n_iter):
            zeros = jax.device_put(
                [np.zeros((n_cores * z.shape[0], *z.shape[1:]), z.dtype)
                 for z in zero_outs], [sharding] * n_outs)
            t0 = time.time()
            outs = sharded(*dev_in, *zeros)
            outs = [np.asarray(o) for o in outs]
            times.append(time.time() - t0)
        per_core = [
            {name: np.split(outs[i], n_cores, axis=0)[c]
             for i, name in enumerate(out_names)}
            for c in range(n_cores)
        ]
        return per_core, times

    run.time_loop = time_loop
    return run


def kernel(x, edge_index, W1, att_src1, att_dst1, b1, W2, att_src2, att_dst2, b2):
    global LAST_EXEC_NS
    cfg = Cfg(N)
    t0 = time.time()
    per_core, meta = preprocess(cfg, edge_index)
    consts = make_consts(cfg, W1, att_src1, att_dst1, b1, W2, att_src2, att_dst2, b2)
    t1 = time.time()
    nc = build(cfg, meta)
    t2 = time.time()
    in_maps = make_inputs(cfg, x, per_core, consts)
    runner = _make_runner(nc, cfg.NCORES)
    n_iter = int(os.environ.get("GAT_TIME_ITERS", "1"))
    results, times = runner(in_maps, n_iter=1)
    t3 = time.time()
    if os.environ.get("GAT_VERBOSE"):
        print(f"[kernel] preprocess {t1-t0:.2f}s build {t2-t1:.2f}s run {t3-t2:.2f}s")
        print(f"[kernel] per-call wall times: {[f'{x*1e3:.2f}ms' for x in times]}")
    if n_iter > 1:
        LAST_EXEC_NS = runner.time_loop(in_maps, R=n_iter) * 1e9
    out = postprocess(cfg, results)
    return np.ascontiguousarray(out.astype(np.float32))


# revision 36
# speedup vs baseline: 1.0839x; 1.0839x over previous
"""Self-contained Trainium2 Bass kernel for the 2-layer GAT problem
(nn_GAT_68264210202658). v2: 4 SWDGE queues, no dst-side gathers (St
transpose trick), attention vectors folded into weights on host."""
import sys, os, time
for p in ('/opt/trn_rl_repo', '/root/.axon_site/_ro/trn_rl_repo', '/root/problem'):
    if p not in sys.path and os.path.isdir(p):
        sys.path.insert(0, p)
import numpy as np
import ml_dtypes
import concourse.bass as bass
import concourse.bacc as bacc
import concourse.mybir as mybir
from concourse import tile

F32 = mybir.dt.float32
BF16 = mybir.dt.bfloat16
I16 = mybir.dt.int16
AF = mybir.ActivationFunctionType
ALU = mybir.AluOpType
NEG_SLOPE = 0.2
NQUEUES = 4


def _r128(x):
    return (int(x) + 127) // 128 * 128


def wrap16(idx):
    """dma_gather idx layout: idx i at [i%16, i//16], replicated to 128 rows."""
    idx = np.asarray(idx, np.int16)
    n = len(idx)
    assert n % 16 == 0
    blk = idx.reshape(n // 16, 16).T  # [16, n/16]
    return np.tile(blk, (8, 1))       # [128, n/16]


class Cfg:
    def __init__(self, N, NCORES=8, IN_C=128, HID=32, HEADS=8, OUT=16):
        assert N % NCORES == 0
        self.N, self.NCORES = N, NCORES
        self.IN_C, self.HID, self.HEADS, self.OUT = IN_C, HID, HEADS, OUT
        self.HC = HID * HEADS                      # 256
        self.NLOC = N // NCORES                    # 6250
        self.NLOCP = _r128(self.NLOC)              # 6272
        self.W = self.NLOCP // 128                 # 49 windows per core
        self.NTOT = self.NLOCP * NCORES            # 50176
        # src side split (512-aligned so phase-A write batches never straddle)
        self.SPLIT = (N // 2) // 512 * 512
        assert 0 < self.SPLIT <= 32767 and N - self.SPLIT <= 32767
        # l2 table split = l2idx(SPLIT); l2idx is monotonic in src
        self.L2SPLIT = (self.SPLIT // self.NLOC) * self.NLOCP + self.SPLIT % self.NLOC
        assert self.L2SPLIT <= 32768 and self.NTOT - self.L2SPLIT <= 32767
        # node table row: [h1 (HC) | a_src1 (HEADS) | pad] in bf16, 256B-mult
        self.ROW1 = _r128(self.HC + self.HEADS)    # 384 bf16 = 768B
        self.ROW1W = self.HC + self.HEADS          # written cols
        self.ROW2 = 64                             # l2 table row (f32): [h2(16)|a_src2|pad]
        self.EXT = self.HC + 2 * self.HEADS        # 272: h1|asrc|adst


def preprocess(cfg, edge_index):
    """Bucket/pad edges. Returns per-core input dict pieces + shared meta."""
    c = cfg
    src = np.asarray(edge_index[0], np.int64)
    dst = np.asarray(edge_index[1], np.int64)
    loops = np.arange(c.N, dtype=np.int64)
    src = np.concatenate([src, loops])
    dst = np.concatenate([dst, loops])

    core = dst // c.NLOC
    dloc = dst - core * c.NLOC
    win = dloc // 128

    # per (core, window, side) edge lists
    buckets = {}
    for cc in range(c.NCORES):
        m = core == cc
        s_c, dl_c, w_c = src[m], dloc[m], win[m]
        sideB = s_c >= c.SPLIT
        for w in range(c.W):
            mw = w_c == w
            for sb in (False, True):
                mm = mw & (sideB == sb)
                buckets[(cc, w, sb)] = (s_c[mm], dl_c[mm])

    NA = [
        _r128(max(len(buckets[(cc, w, False)][0]) for cc in range(c.NCORES)))
        for w in range(c.W)
    ]
    NB = [
        _r128(max(len(buckets[(cc, w, True)][0]) for cc in range(c.NCORES)))
        for w in range(c.W)
    ]
    # avoid zero-size gathers: keep >=128 so program structure is sane
    NA = [max(n, 128) for n in NA]
    NB = [max(n, 128) for n in NB]

    # batches of 2 windows; group ordering per batch: [A_w0 | A_w1 | B_w0 | B_w1]
    batches = [tuple(range(i, min(i + 2, c.W))) for i in range(0, c.W, 2)]
    meta = {"NA": NA, "NB": NB, "batches": batches}

    # global group numbering
    gof = {}   # (w, side) -> first group index
    g = 0
    for ws in batches:
        for w in ws:
            gof[(w, 0)] = g
            g += NA[w] // 128
        for w in ws:
            gof[(w, 1)] = g
            g += NB[w] // 128
    GT = g
    meta["gof"] = gof
    meta["GT"] = GT

    def l2idx(s):
        return (s // c.NLOC) * c.NLOCP + (s % c.NLOC)

    per_core = []
    for cc in range(c.NCORES):
        iA, iB, iA2, iB2 = [], [], [], []
        dl_cols = np.full((128, GT), -1.0, np.float32)
        for ws in batches:
            for side in (0, 1):
                for w in ws:
                    s_e, dl_e = buckets[(cc, w, bool(side))]
                    n = (NA if side == 0 else NB)[w]
                    npad = n - len(s_e)
                    padv = 0 if side == 0 else c.SPLIT
                    sp = np.concatenate([s_e, np.full(npad, padv, np.int64)])
                    dlp = np.concatenate(
                        [dl_e % 128, np.full(npad, -1, np.int64)]
                    ).astype(np.float32)
                    if side == 0:
                        iA.append(sp)
                        iA2.append(l2idx(sp))
                    else:
                        iB.append(sp - c.SPLIT)
                        iB2.append(l2idx(sp) - c.L2SPLIT)
                    g0 = gof[(w, side)]
                    dl_cols[:, g0 : g0 + n // 128] = dlp.reshape(n // 128, 128).T
        iA = np.concatenate(iA)
        iB = np.concatenate(iB)
        iA2 = np.concatenate(iA2)
        iB2 = np.concatenate(iB2)
        per_core.append(
            {
                "idxA": wrap16(iA),
                "idxB": wrap16(iB),
                "idxA2": wrap16(iA2),
                "idxB2": wrap16(iB2),
                "dl": dl_cols.astype(ml_dtypes.bfloat16),
            }
        )
    return per_core, meta


def make_consts(cfg, W1, att_src1, att_dst1, b1, W2, att_src2, att_dst2, b2):
    c = cfg
    bf = ml_dtypes.bfloat16
    H, HID, HC, OUT = c.HEADS, c.HID, c.HC, c.OUT
    W1 = np.asarray(W1, np.float32)
    As = np.zeros((HC, H), np.float32)
    Ad = np.zeros((HC, H), np.float32)
    a_s1 = np.asarray(att_src1, np.float32)
    a_d1 = np.asarray(att_dst1, np.float32)
    for h in range(H):
        As[h * HID : (h + 1) * HID, h] = a_s1[h]
        Ad[h * HID : (h + 1) * HID, h] = a_d1[h]
    W1e = np.concatenate([W1, W1 @ As, W1 @ Ad], axis=1)       # [128, 272]
    w2 = np.asarray(W2, np.float32)                            # [256, 16]
    W2e = np.concatenate(
        [w2,
         w2 @ np.asarray(att_src2, np.float32).reshape(OUT, 1),
         w2 @ np.asarray(att_dst2, np.float32).reshape(OUT, 1)],
        axis=1,
    )                                                          # [256, 18]
    cb = np.concatenate(
        [W1e] + [W2e[k * 128 : (k + 1) * 128, :] for k in range(HC // 128)],
        axis=1,
    ).astype(bf)                                               # [128, 272+2*18]
    rep = lambda v: np.tile(np.asarray(v, np.float32).reshape(1, -1), (128, 1))
    cf = np.concatenate([rep(b1), rep(b2)], axis=1).astype(np.float32)
    return cb, cf


def build(cfg, meta, phases='ABCDE', debug=False):
    c = cfg
    NA, NB, batches, gof = meta["NA"], meta["NB"], meta["batches"], meta["gof"]
    GT = meta["GT"]
    H, HID, HC, OUT = c.HEADS, c.HID, c.HC, c.OUT
    KF = c.HC // 128  # feature chunks for layer-2 matmul (2)
    EXT = c.EXT       # 272
    O2 = OUT + 2      # 18: h2|asrc2|adst2

    nc = bacc.Bacc(None, target_bir_lowering=False, debug=False,
                   num_swdge_queues=NQUEUES)

    xT = nc.dram_tensor("xT", [128, c.NTOT], BF16, kind="ExternalInput")
    xTL = nc.dram_tensor("xTL", [128, c.NLOCP], BF16, kind="ExternalInput")
    cbf = nc.dram_tensor("cbf", [128, EXT + KF * O2], BF16, kind="ExternalInput")
    cf32 = nc.dram_tensor("cf32", [128, HC + OUT], F32, kind="ExternalInput")
    sumA, sumB = sum(NA), sum(NB)
    idxA = nc.dram_tensor("idxA", [128, sumA // 16], I16, kind="ExternalInput")
    idxB = nc.dram_tensor("idxB", [128, sumB // 16], I16, kind="ExternalInput")
    idxA2 = nc.dram_tensor("idxA2", [128, sumA // 16], I16, kind="ExternalInput")
    idxB2 = nc.dram_tensor("idxB2", [128, sumB // 16], I16, kind="ExternalInput")
    dlin = nc.dram_tensor("dl", [128, GT], BF16, kind="ExternalInput")
    out = nc.dram_tensor("out", [c.NLOCP, OUT], F32, kind="ExternalOutput")

    htabA = nc.dram_tensor("htabA", [c.SPLIT, c.ROW1], BF16)
    htabB = nc.dram_tensor("htabB", [c.NTOT - c.SPLIT, c.ROW1], BF16)
    l2loc = nc.dram_tensor("l2loc", [c.NLOCP, c.ROW2], F32)
    l2glob = nc.dram_tensor("l2glob", [c.NTOT, c.ROW2], F32,
                            addr_space="Shared")
    l2globB = nc.dram_tensor("l2globB", [c.NTOT - c.L2SPLIT, c.ROW2], F32)
    if debug:
        ndbg = min(c.SPLIT, c.NLOCP)
        dbgH = nc.dram_tensor("dbgH", [ndbg, c.ROW1W], BF16, kind="ExternalOutput")
        dbgL = nc.dram_tensor("dbgL", [c.NLOCP, OUT + 1], F32, kind="ExternalOutput")
        dbgD2 = nc.dram_tensor("dbgD2", [c.NLOCP, 1], BF16, kind="ExternalOutput")

    ctx_lp = nc.allow_low_precision("bf16 tables are intentional")
    ctx_lp.__enter__()
    with tile.TileContext(nc) as tc:
        with tc.tile_pool(name="consts", bufs=1) as cpool:
            CB = cpool.tile([128, EXT + KF * O2], BF16)
            CF = cpool.tile([128, HC + OUT], F32)
            nc.sync.dma_start(CB[:], cbf[:])
            nc.sync.dma_start(CF[:], cf32[:])
            W1e = CB[:, 0:EXT]
            W2s = [CB[:, EXT + k * O2 : EXT + (k + 1) * O2] for k in range(KF)]
            B1 = CF[:, 0:HC]
            B2 = CF[:, HC : HC + OUT]

            iota_i = cpool.tile([128, 128], I16)
            nc.gpsimd.iota(iota_i[:], [[1, 128]], channel_multiplier=0)
            iota_b = cpool.tile([128, 128], BF16)
            nc.vector.tensor_copy(iota_b[:], iota_i[:])
            iotac_i = cpool.tile([128, 1], I16)
            nc.gpsimd.iota(iotac_i[:], [[1, 1]], channel_multiplier=1)
            iotac_f = cpool.tile([128, 1], F32)
            nc.vector.tensor_copy(iotac_f[:], iotac_i[:])
            iota_f = cpool.tile([128, 128], F32)
            nc.vector.tensor_copy(iota_f[:], iota_i[:])
            ident = cpool.tile([128, 128], BF16)
            nc.vector.tensor_scalar(ident[:], iota_f[:], iotac_f[:], None, ALU.is_equal)

            AD1sb = cpool.tile([128, c.W, H], BF16)   # a_dst1 per (lane, win)
            dst2_acc = cpool.tile([128, c.W], BF16)   # a_dst2 per (lane, win)

            def gather_chunked(out_tile, g_off, table, idx_tile, i_off, n, elem):
                """dma_gather in <=1024-idx chunks. queue_num is reassigned
                post-compile to match the scheduler's DMASW lane striping."""
                done = 0
                while done < n:
                    k = min(1024, n - done)
                    nc.gpsimd.dma_gather(
                        out_tile[:, (g_off + done) // 128 : (g_off + done + k) // 128, :],
                        table[:, :], idx_tile[:, (i_off + done) // 16 : (i_off + done + k) // 16],
                        k, k, elem, queue_num=0)
                    done += k

            # ---------------- Phase A: global h1|asrc table ----------------
            TB = 4  # node tiles per batch
            nbat = c.NTOT // (128 * TB) if 'A' in phases else 0
            assert c.NTOT % (128 * TB) == 0
            with (
                tc.tile_pool(name="pa", bufs=3) as pa,
                tc.tile_pool(name="ppa", bufs=2, space="PSUM") as ppa,
            ):
                def node_tile_batch(xsrc, t0, nt):
                    xt_ = pa.tile([128, nt, 128], BF16, tag="xt")
                    nc.sync.dma_start(
                        xt_[:], xsrc[:, 128 * t0 : 128 * (t0 + nt)]
                        .rearrange("p (g n) -> p g n", g=nt)
                    )
                    # 512-f32 group stride: matmul outputs (272 f32 = 1088B)
                    # must not straddle 2KB PSUM bank boundaries
                    hp_ = ppa.tile([128, nt, 512], F32, tag="h1p")
                    for j in range(nt):
                        nc.tensor.matmul(
                            hp_[:, j, 0:EXT], xt_[:, j, :], W1e,
                            start=True, stop=True,
                        )
                    return hp_

                for b in range(nbat):
                    hp_ = node_tile_batch(xT, b * TB, TB)
                    row = pa.tile([128, TB, c.ROW1W], BF16, tag="row")
                    if b % 2 == 0:
                        nc.scalar.copy(row[:], hp_[:, :, 0 : c.ROW1W])
                    else:
                        nc.vector.tensor_copy(row[:], hp_[:, :, 0 : c.ROW1W])
                    r0 = 128 * TB * b
                    r1 = 128 * TB * (b + 1)
                    assert r1 <= c.SPLIT or r0 >= c.SPLIT, "batch straddles SPLIT"
                    tgt = (htabA[r0:r1, 0:c.ROW1W] if r1 <= c.SPLIT
                           else htabB[r0 - c.SPLIT : r1 - c.SPLIT, 0:c.ROW1W])
                    nc.sync.dma_start(
                        tgt.rearrange("(g p) c -> p g c", p=128), row[:])
                # A-local: a_dst1 for own nodes -> SBUF resident
                t = 0 if 'A' in phases else c.W
                while t < c.W:
                    nt = min(TB, c.W - t)
                    hp_ = node_tile_batch(xTL, t, nt)
                    nc.vector.tensor_copy(
                        AD1sb[:, t : t + nt, :], hp_[:, :, HC + H : HC + 2 * H])
                    t += nt

            # ---------------- Phases B + C: layer-1 windows ----------------
            bc_batches = batches if 'B' in phases else []
            oA = oB = 0
            with (
                tc.tile_pool(name="pb", bufs=2) as pb,
                tc.tile_pool(name="pbs", bufs=3) as pbs,
                tc.tile_pool(name="pst", bufs=4) as pst,
                tc.tile_pool(name="ppb", bufs=1, space="PSUM") as ppb,
                tc.tile_pool(name="pptp", bufs=3, space="PSUM") as pptp,
                tc.tile_pool(name="ppad", bufs=1, space="PSUM") as ppad,
                tc.tile_pool(name="ppc", bufs=1, space="PSUM") as ppc,
            ):
                for bi, ws in enumerate(bc_batches):
                    nA = sum(NA[w] for w in ws)
                    nB = sum(NB[w] for w in ws)
                    nE = nA + nB
                    gb = nE // 128          # groups this batch
                    g0 = gof[(ws[0], 0)]    # first global group
                    # group -> window map for this batch
                    g2w = [None] * gb
                    for w in ws:
                        for side in (0, 1):
                            n = (NA if side == 0 else NB)[w]
                            for g in range(gof[(w, side)] - g0,
                                           gof[(w, side)] - g0 + n // 128):
                                g2w[g] = w
                    # idx tiles
                    tA = pb.tile([128, nA // 16], I16, tag="tA")
                    tB = pb.tile([128, nB // 16], I16, tag="tB")
                    nc.sync.dma_start(tA[:], idxA[:, oA // 16 : (oA + nA) // 16])
                    nc.sync.dma_start(tB[:], idxB[:, oB // 16 : (oB + nB) // 16])
                    dl = pb.tile([128, gb], BF16, tag="dl")
                    nc.sync.dma_start(dl[:], dlin[:, g0 : g0 + gb])
                    # gathers (src rows only)
                    G = pb.tile([128, gb, c.ROW1], BF16, tag="G")
                    gather_chunked(G, 0, htabA, tA, 0, nA, c.ROW1)
                    gather_chunked(G, nA, htabB, tB, 0, nB, c.ROW1)
                    # one-hot S for all groups (bf16, one DVE op)
                    S_all = pb.tile([128, gb, 128], BF16, tag="S")
                    nc.vector.tensor_tensor(
                        S_all[:],
                        iota_b[:].unsqueeze(1).broadcast_to([128, gb, 128]),
                        dl[:].unsqueeze(2).broadcast_to([128, gb, 128]),
                        ALU.is_equal)
                    # per-group St = S^T; adst_e = St^T @ AD1sb[win]
                    adst_all = ppad.tile([128, gb, H], F32, tag="adst")
                    for g in range(gb):
                        tp = pptp.tile([128, 128], BF16, tag="tp")
                        nc.tensor.transpose(tp[:], S_all[:, g, :], ident[:])
                        Stg = pst.tile([128, 128], BF16, tag="St")
                        if g % 2 == 0:
                            nc.scalar.copy(Stg[:], tp[:])
                        else:
                            nc.vector.tensor_copy(Stg[:], tp[:])
                        nc.tensor.matmul(
                            adst_all[:, g, :], Stg[:], AD1sb[:, g2w[g], :],
                            start=True, stop=True)
                    # logits
                    lg = pbs.tile([128, gb, H], F32, tag="lg")
                    nc.vector.tensor_add(lg[:], G[:, :, HC : HC + H], adst_all[:])
                    nc.vector.scalar_tensor_tensor(
                        lg[:], lg[:], NEG_SLOPE, lg[:], ALU.mult, ALU.max)
                    ex = pbs.tile([128, gb, H], BF16, tag="ex")
                    nc.scalar.activation(ex[:], lg[:], AF.Exp)
                    # rhs = [msg | ex]
                    rhs = pb.tile([128, gb, HC + H], BF16, tag="rhs")
                    nc.vector.tensor_tensor(
                        rhs[:, :, 0:HC].rearrange("p g (h c) -> p g h c", h=H),
                        G[:, :, 0:HC].rearrange("p g (h c) -> p g h c", h=H),
                        ex[:].unsqueeze(3).broadcast_to([128, gb, H, HID]),
                        ALU.mult)
                    nc.scalar.copy(rhs[:, :, HC : HC + H], ex[:])
                    # per-batch reduce + normalize + elu (both windows at once)
                    nw = len(ws)
                    acc2 = ppb.tile([128, nw, 512], F32, tag="acc")
                    for wi, w in enumerate(ws):
                        glist = (
                            list(range(gof[(w, 0)] - g0, gof[(w, 0)] - g0 + NA[w] // 128))
                            + list(range(gof[(w, 1)] - g0, gof[(w, 1)] - g0 + NB[w] // 128))
                        )
                        for i, g in enumerate(glist):
                            nc.tensor.matmul(
                                acc2[:, wi, 0 : HC + H], S_all[:, g, :], rhs[:, g, :],
                                start=(i == 0), stop=(i == len(glist) - 1))
                    den = pbs.tile([128, nw, H], F32, tag="den")
                    nc.vector.tensor_scalar(
                        den[:], acc2[:, :, HC : HC + H], 1e-30, None, ALU.max)
                    rec = pbs.tile([128, nw, H], F32, tag="rec")
                    nc.vector.reciprocal(rec[:], den[:])
                    o1 = pbs.tile([128, nw, HC], F32, tag="o1")
                    nc.vector.tensor_tensor(
                        o1[:].rearrange("p w (h c) -> p w h c", h=H),
                        acc2[:, :, 0:HC].rearrange("p w (h c) -> p w h c", h=H),
                        rec[:].unsqueeze(3).broadcast_to([128, nw, H, HID]),
                        ALU.mult)
                    nc.vector.tensor_add(
                        o1[:], o1[:], B1.unsqueeze(1).broadcast_to([128, nw, HC]))
                    # elu: hp = max(o1,0) + exp(min(o1,0)) - 1
                    t1 = pbs.tile([128, nw, HC], F32, tag="t1")
                    nc.vector.tensor_scalar(t1[:], o1[:], 0.0, None, ALU.min)
                    e1 = pbs.tile([128, nw, HC], F32, tag="e1")
                    nc.scalar.activation(e1[:], t1[:], AF.Exp)
                    nc.vector.tensor_scalar(t1[:], o1[:], 0.0, None, ALU.max)
                    hp = pbs.tile([128, nw, HC], BF16, tag="hp")
                    nc.vector.scalar_tensor_tensor(
                        hp[:], e1[:], -1.0, t1[:], ALU.add, ALU.add)
                    # ---- phase C for this batch ----
                    if 'C' in phases:
                        for wi, w in enumerate(ws):
                            tp2 = ppc.tile([128, KF, 128], BF16, tag="tp2")
                            for k in range(KF):
                                nc.tensor.transpose(
                                    tp2[:, k, :],
                                    hp[:, wi, 128 * k : 128 * (k + 1)], ident[:])
                            tpb = pbs.tile([128, KF, 128], BF16, tag="tpb")
                            nc.scalar.copy(tpb[:], tp2[:])
                            h2p = ppc.tile([128, O2], F32, tag="h2p")
                            for k in range(KF):
                                nc.tensor.matmul(
                                    h2p[:], tpb[:, k, :], W2s[k],
                                    start=(k == 0), stop=(k == KF - 1))
                            row2 = pbs.tile([128, OUT + 1], F32, tag="row2")
                            nc.scalar.copy(row2[:], h2p[:, 0 : OUT + 1])
                            nc.sync.dma_start(
                                l2loc[128 * w : 128 * (w + 1), 0 : OUT + 1]
                                .rearrange("(g p) c -> p g c", p=128)[:, 0, :],
                                row2[:])
                            nc.vector.tensor_copy(
                                dst2_acc[:, w : w + 1], h2p[:, OUT + 1 : OUT + 2])
                    oA += nA
                    oB += nB

            if debug:
                nc.sync.dma_start(dbgH[:], htabA[0:ndbg, 0:c.ROW1W])
                nc.sync.dma_start(dbgL[:], l2loc[:, 0 : OUT + 1])
                nc.sync.dma_start(
                    dbgD2[:].rearrange("(w p) c -> p w c", p=128),
                    dst2_acc[:].unsqueeze(2))

            # ---------------- Phase D: allgather l2 ----------------
            if 'D' in phases:
                nc.gpsimd.collective_compute(
                    "AllGather", ALU.bypass,
                    replica_groups=[list(range(c.NCORES))],
                    ins=[l2loc[:].opt()], outs=[l2glob[:].opt()])
            elif 'E' in phases:
                # debug: bypass collective, local copy only (wrong values off-core)
                nc.sync.dma_start(l2glob[0 : c.NLOCP, :], l2loc[:])
            if 'D' in phases or 'E' in phases:
                nc.sync.dma_start(l2globB[:], l2glob[c.L2SPLIT : c.NTOT, :])

            # ---------------- Phase E: layer-2 windows ----------------
            oA = oB = 0
            e_batches = batches if 'E' in phases else []
            with (
                tc.tile_pool(name="pe", bufs=2) as pe,
                tc.tile_pool(name="pes", bufs=3) as pes,
                tc.tile_pool(name="pst2", bufs=4) as pst2,
                tc.tile_pool(name="ppe", bufs=2, space="PSUM") as ppe,
                tc.tile_pool(name="pptp2", bufs=3, space="PSUM") as pptp2,
                tc.tile_pool(name="ppad2", bufs=2, space="PSUM") as ppad2,
            ):
                for ws in e_batches:
                    nA = sum(NA[w] for w in ws)
                    nB = sum(NB[w] for w in ws)
                    nE = nA + nB
                    gb = nE // 128
                    g0 = gof[(ws[0], 0)]
                    g2w = [None] * gb
                    for w in ws:
                        for side in (0, 1):
                            n = (NA if side == 0 else NB)[w]
                            for g in range(gof[(w, side)] - g0,
                                           gof[(w, side)] - g0 + n // 128):
                                g2w[g] = w
                    tA = pe.tile([128, nA // 16], I16, tag="tA")
                    tB = pe.tile([128, nB // 16], I16, tag="tB")
                    nc.sync.dma_start(tA[:], idxA2[:, oA // 16 : (oA + nA) // 16])
                    nc.sync.dma_start(tB[:], idxB2[:, oB // 16 : (oB + nB) // 16])
                    dl = pe.tile([128, gb], BF16, tag="dl")
                    nc.sync.dma_start(dl[:], dlin[:, g0 : g0 + gb])
                    G2 = pe.tile([128, gb, c.ROW2], F32, tag="G2")
                    gather_chunked(G2, 0, l2glob[0 : c.L2SPLIT, :], tA, 0, nA, c.ROW2)
                    gather_chunked(G2, nA, l2globB, tB, 0, nB, c.ROW2)
                    S_all = pe.tile([128, gb, 128], BF16, tag="S2")
                    nc.vector.tensor_tensor(
                        S_all[:],
                        iota_b[:].unsqueeze(1).broadcast_to([128, gb, 128]),
                        dl[:].unsqueeze(2).broadcast_to([128, gb, 128]),
                        ALU.is_equal)
                    adst_all = ppad2.tile([128, gb, 1], F32, tag="adst2")
                    for g in range(gb):
                        tp = pptp2.tile([128, 128], BF16, tag="tp")
                        nc.tensor.transpose(tp[:], S_all[:, g, :], ident[:])
                        Stg = pst2.tile([128, 128], BF16, tag="St")
                        if g % 2 == 0:
                            nc.scalar.copy(Stg[:], tp[:])
                        else:
                            nc.vector.tensor_copy(Stg[:], tp[:])
                        nc.tensor.matmul(
                            adst_all[:, g, :], Stg[:],
                            dst2_acc[:, g2w[g] : g2w[g] + 1],
                            start=True, stop=True)
                    lg = pes.tile([128, gb, 1], F32, tag="lg2")
                    nc.vector.tensor_add(
                        lg[:], G2[:, :, OUT : OUT + 1], adst_all[:])
                    nc.vector.scalar_tensor_tensor(
                        lg[:], lg[:], NEG_SLOPE, lg[:], ALU.mult, ALU.max)
                    ex = pes.tile([128, gb, 1], BF16, tag="ex2")
                    nc.scalar.activation(ex[:], lg[:], AF.Exp)
                    rhs = pe.tile([128, gb, OUT + 1], BF16, tag="rhs2")
                    nc.vector.tensor_tensor(
                        rhs[:, :, 0:OUT], G2[:, :, 0:OUT],
                        ex[:].broadcast_to([128, gb, OUT]), ALU.mult)
                    nc.scalar.copy(rhs[:, :, OUT : OUT + 1], ex[:])
                    nw = len(ws)
                    acc2 = ppe.tile([128, nw, 32], F32, tag="acc2")
                    for wi, w in enumerate(ws):
                        glist = (
                            list(range(gof[(w, 0)] - g0, gof[(w, 0)] - g0 + NA[w] // 128))
                            + list(range(gof[(w, 1)] - g0, gof[(w, 1)] - g0 + NB[w] // 128))
                        )
                        for i, g in enumerate(glist):
                            nc.tensor.matmul(
                                acc2[:, wi, 0 : OUT + 1], S_all[:, g, :], rhs[:, g, :],
                                start=(i == 0), stop=(i == len(glist) - 1))
                    den = pes.tile([128, nw, 1], F32, tag="den2")
                    nc.vector.tensor_scalar(
                        den[:], acc2[:, :, OUT : OUT + 1], 1e-30, None, ALU.max)
                    rec = pes.tile([128, nw, 1], F32, tag="rec2")
                    nc.vector.reciprocal(rec[:], den[:])
                    o2 = pes.tile([128, nw, OUT], F32, tag="o2")
                    nc.vector.tensor_tensor(
                        o2[:], acc2[:, :, 0:OUT],
                        rec[:].broadcast_to([128, nw, OUT]), ALU.mult)
                    nc.vector.tensor_add(
                        o2[:], o2[:], B2.unsqueeze(1).broadcast_to([128, nw, OUT]))
                    nc.sync.dma_start(
                        out[128 * ws[0] : 128 * (ws[0] + nw), :]
                        .rearrange("(g p) c -> p g c", p=128),
                        o2[:])
                    oA += nA
                    oB += nB
    ctx_lp.__exit__(None, None, None)
    nc.compile()
    # Stripe SWDGE queues to match the tile scheduler's DMASW-lane round-robin
    # (lane = i % 8 over Pool DMA insts in scheduled order; each lane must
    # always pair with one queue, so queue = lane % NQUEUES = i % NQUEUES).
    from concourse.tile_scheduler import DMAInst
    i = 0
    for bb in nc.m.functions[0].blocks:
        for inst in bb.instructions:
            if isinstance(inst, DMAInst) and inst.engine == mybir.EngineType.Pool:
                inst.queue_num = i % NQUEUES
                i += 1
    return nc


def make_inputs(cfg, x, per_core, consts):
    c = cfg
    bf = ml_dtypes.bfloat16
    cb, cf = consts
    xTfull = np.zeros((128, c.NTOT), bf)
    xTfull[:, 0 : c.N] = np.ascontiguousarray(np.asarray(x, np.float32).T).astype(bf)
    in_maps = []
    for cc in range(c.NCORES):
        xtl = np.zeros((128, c.NLOCP), bf)
        xtl[:, 0 : c.NLOC] = xTfull[:, cc * c.NLOC : (cc + 1) * c.NLOC]
        m = per_core[cc]
        in_maps.append(
            {
                "xT": xTfull,
                "xTL": xtl,
                "cbf": cb,
                "cf32": cf,
                "idxA": m["idxA"],
                "idxB": m["idxB"],
                "idxA2": m["idxA2"],
                "idxB2": m["idxB2"],
                "dl": m["dl"],
            }
        )
    return in_maps


def postprocess(cfg, results):
    c = cfg
    outs = [results[cc]["out"][0 : c.NLOC, :] for cc in range(c.NCORES)]
    return np.concatenate(outs, axis=0)


LAST_EXEC_NS = None
N = 50000


def _make_runner(nc, n_cores):
    """Build the shard_map-jitted executable once (mirrors
    bass2jax.run_bass_via_pjrt) so repeated timed calls skip re-tracing."""
    import jax
    from jax.experimental.shard_map import shard_map
    from jax.sharding import Mesh, PartitionSpec
    from concourse import bass2jax, mybir
    from concourse.bass2jax import _bass_exec_p, partition_id_tensor, install_neuronx_cc_hook

    install_neuronx_cc_hook()
    partition_name = nc.partition_id_tensor.name if nc.partition_id_tensor else None
    in_names, out_names, out_avals, zero_outs, in_structs = [], [], [], [], []
    for alloc in nc.m.functions[0].allocations:
        if not isinstance(alloc, mybir.MemoryLocationSet):
            continue
        name = alloc.memorylocations[0].name
        if alloc.kind == "ExternalInput":
            if name != partition_name:
                in_names.append(name)
                in_structs.append(
                    (tuple(alloc.tensor_shape), mybir.dt.np(alloc.dtype)))
        elif alloc.kind == "ExternalOutput":
            out_names.append(name)
            shape = tuple(alloc.tensor_shape)
            dtype = mybir.dt.np(alloc.dtype)
            out_avals.append(jax.core.ShapedArray(shape, dtype))
            zero_outs.append(np.zeros(shape, dtype))
    n_params = len(in_names)
    n_outs = len(out_avals)
    all_in = list(in_names) + list(out_names)
    if partition_name is not None:
        all_in.append(partition_name)
    donate = tuple(range(n_params, n_params + n_outs))

    def _body(*args):
        operands = list(args)
        if partition_name is not None:
            operands.append(partition_id_tensor())
        return tuple(
            _bass_exec_p.bind(
                *operands,
                out_avals=tuple(out_avals),
                in_names=tuple(all_in),
                out_names=tuple(out_names),
                lowering_input_output_aliases=(),
                sim_require_finite=False,
                sim_require_nnan=False,
                nc=nc,
            )
        )

    devices = jax.devices()[:n_cores]
    mesh = Mesh(np.asarray(devices), ("core",))
    in_specs = (PartitionSpec("core"),) * (n_params + n_outs)
    out_specs = (PartitionSpec("core"),) * n_outs
    sharding = jax.sharding.NamedSharding(mesh, PartitionSpec("core"))

    def _compile():
        jt = jax.jit(
            shard_map(_body, mesh=mesh, in_specs=in_specs, out_specs=out_specs,
                      check_rep=False),
            donate_argnums=donate, keep_unused=True)
        structs = [
            jax.ShapeDtypeStruct((n_cores * s[0], *s[1:]), dt, sharding=sharding)
            for s, dt in in_structs
        ] + [
            jax.ShapeDtypeStruct((n_cores * z.shape[0], *z.shape[1:]), z.dtype,
                                 sharding=sharding)
            for z in zero_outs
        ]
        return jt.lower(*structs).compile()

    try:
        from concourse.bass2jax import fast_dispatch_compile
        sharded = fast_dispatch_compile(_compile)
    except Exception as e:
        print(f"[kernel] fast_dispatch_compile unavailable ({e}); plain jit")
        sharded = jax.jit(
            shard_map(_body, mesh=mesh, in_specs=in_specs, out_specs=out_specs,
                      check_rep=False),
            donate_argnums=donate, keep_unused=True)

    def time_loop(in_maps, R=30):
        """Chain R executions, feeding each call's donated outputs back in
        (device-resident) so per-iteration cost ~= device exec + dispatch.
        Warm up with the same call signature as the timed loop so jax
        retracing / first-donation setup stays out of the timed window."""
        import jax
        concat_in = [
            np.concatenate([np.asarray(in_maps[c][i_name]) for c in range(n_cores)], axis=0)
            for i_name in in_names
        ]
        dev_in = jax.device_put(concat_in, [sharding] * n_params)
        zeros = jax.device_put(
            [np.zeros((n_cores * z.shape[0], *z.shape[1:]), z.dtype)
             for z in zero_outs], [sharding] * n_outs)
        outs = sharded(*dev_in, *zeros)
        jax.block_until_ready(outs)
        # same-signature warmups (device-resident donated outs)
        for _ in range(3):
            outs = sharded(*dev_in, *outs)
            jax.block_until_ready(outs)
        t0 = time.time()
        for _ in range(R):
            outs = sharded(*dev_in, *outs)
        jax.block_until_ready(outs)
        return (time.time() - t0) / R

    def run(in_maps, n_iter=1):
        import jax
        concat_in = [
            np.concatenate([np.asarray(in_maps[c][i_name]) for c in range(n_cores)], axis=0)
            for i_name in in_names
        ]
        dev_in = jax.device_put(concat_in, [sharding] * n_params)
        times = []
        outs = None
        for _ in range(n_iter):
            zeros = jax.device_put(
                [np.zeros((n_cores * z.shape[0], *z.shape[1:]), z.dtype)
                 for z in zero_outs], [sharding] * n_outs)
            t0 = time.time()
            outs = sharded(*dev_in, *zeros)
            outs = [np.asarray(o) for o in outs]
            times.append(time.time() - t0)
        per_core = [
            {name: np.split(outs[i], n_cores, axis=0)[c]
             for i, name in enumerate(out_names)}
            for c in range(n_cores)
        ]
        return per_core, times

    run.time_loop = time_loop
    return run


def kernel(x, edge_index, W1, att_src1, att_dst1, b1, W2, att_src2, att_dst2, b2):
    global LAST_EXEC_NS
    cfg = Cfg(N)
    t0 = time.time()
    per_core, meta = preprocess(cfg, edge_index)
    consts = make_consts(cfg, W1, att_src1, att_dst1, b1, W2, att_src2, att_dst2, b2)
    t1 = time.time()
    nc = build(cfg, meta)
    t2 = time.time()
    in_maps = make_inputs(cfg, x, per_core, consts)
    runner = _make_runner(nc, cfg.NCORES)
    n_iter = int(os.environ.get("GAT_TIME_ITERS", "1"))
    results, times = runner(in_maps, n_iter=1)
    t3 = time.time()
    if os.environ.get("GAT_VERBOSE"):
        print(f"[kernel] preprocess {t1-t0:.2f}s build {t2-t1:.2f}s run {t3-t2:.2f}s")
        print(f"[kernel] per-call wall times: {[f'{x*1e3:.2f}ms' for x in times]}")
    if n_iter > 1:
        LAST_EXEC_NS = runner.time_loop(in_maps, R=n_iter) * 1e9
    out = postprocess(cfg, results)
    return np.ascontiguousarray(out.astype(np.float32))
